# revision 1
# baseline (speedup 1.0000x reference)
"""Trainium2 Bass kernel for nn_Attention_KV (dense transformer attention
with K=Q sharing and a linear positional bias), distributed over 8 cores.

Sharding: 2 batch-groups x 4 query-quarters (collective-free). Core
c = 4*g + s owns batches 4g..4g+3 and query rows i in [256*s, 256*(s+1)).
The positional bias pos_bias(i,j) is head/batch independent but sharded
by i-quarter, so each core loads exactly the pos slice it consumes —
no AllGather (measured ~300us fixed latency per collective on this
fabric, more than the whole rest of the kernel). The price is computing
k/v for 4 batches per core (cheap PE work) instead of 1.

All attention math keeps scores TRANSPOSED (keys j on partitions,
queries i on the free axis). Because dots = k @ k^T is symmetric this
costs nothing, and it makes softmax + the attn @ v contraction
expressible without any on-chip transpose:
  - scores^T lands directly in PSUM: a K=64 dots matmul plus an identity
    matmul that adds pos_bias^T/c (pos is pre-divided by
    c = scale*sum(w_pos) on-device, so exp(scale=c) on the Scalar engine
    applies both the dot-product scaling and the bias in one pass)
  - attn@v as lhsT = v_ext (with a ones column appended -> row 64 of the
    result is the softmax denominator Z), rhs = exp(scores^T)
  - normalization folded into the PSUM->SBUF copy of U
pos flows in bf16 (softmax averaging damps its rounding error to ~1e-4
relative on the output); matmuls run in float32r. b_pos (a scalar added
to every score) is dropped: softmax is shift invariant.
"""

import sys

sys.path.insert(0, "/opt/trn_rl_repo")

import numpy as np

import concourse.bacc as bacc
import concourse.bass as bass
import concourse.mybir as mybir
from concourse import tile
from concourse.bass_utils import run_bass_kernel_spmd

B, N, DIM, H, POS_DIM = 8, 1024, 512, 8, 50
D = DIM // H  # 64
NC = 8  # cores
BPC = 4  # batches per core
IQ = 256  # query rows per core
JT = N // 128  # 8 j-tiles
SCALE = float(DIM) ** -0.5

F32 = mybir.dt.float32
F32R = mybir.dt.float32r
BF16 = mybir.dt.bfloat16
AX = mybir.AxisListType
ALU = mybir.AluOpType
ACTF = mybir.ActivationFunctionType

POS_CHUNK = 64  # i-columns of pos processed per DVE reduce


def build_program(reps: int = 1):
    nc = bacc.Bacc("TRN2", target_bir_lowering=False, debug=False)

    # ---- DRAM parameters (per-core) ----
    xT_d = nc.declare_dram_parameter("xT", [BPC, DIM, N], F32R, isOutput=False)
    xqT_d = nc.declare_dram_parameter("xqT", [BPC, DIM, IQ], F32R, isOutput=False)
    wkvT_d = nc.declare_dram_parameter("wkvT", [DIM, 2 * DIM], F32R, isOutput=False)
    wout_d = nc.declare_dram_parameter("wout", [DIM, DIM], F32R, isOutput=False)
    bout_d = nc.declare_dram_parameter("bout", [1, DIM], F32R, isOutput=False)
    wposr_d = nc.declare_dram_parameter(
        "wposr", [128, POS_CHUNK, POS_DIM], BF16, isOutput=False
    )
    posT_d = nc.declare_dram_parameter("posT", [N, IQ, POS_DIM], BF16, isOutput=False)
    ones_d = nc.declare_dram_parameter("ones", [128, 128], F32R, isOutput=False)
    id_d = nc.declare_dram_parameter("idm", [128, 128], BF16, isOutput=False)
    y_d = nc.declare_dram_parameter("y", [BPC, IQ, DIM], F32, isOutput=True)

    with tile.TileContext(nc) as tc:
        with (
            tc.tile_pool(name="persist", bufs=1) as pp,
            tc.tile_pool(name="pos_in", bufs=2) as pos_pool,
            tc.tile_pool(name="exps", bufs=3) as epool,
            tc.tile_pool(name="outsb", bufs=2) as opool,
            tc.tile_pool(name="mm_ps", bufs=2, space="PSUM") as mmps,
            tc.tile_pool(name="dots_ps", bufs=2, space="PSUM") as dotsps,
            tc.tile_pool(name="up_ps", bufs=2, space="PSUM") as upps,
            tc.tile_pool(name="dram", bufs=1, space="DRAM") as dram,
        ):
            for _rep in range(reps):
                # ---- preload small tensors + weights ----
                wposr = pp.tile([128, POS_CHUNK, POS_DIM], BF16, tag="wposr")
                nc.sync.dma_start(wposr[:], wposr_d[:])
                ones1 = pp.tile([1, 128], F32R, tag="ones1")
                nc.sync.dma_start(ones1[:], ones_d[0:1, :])
                idm = pp.tile([128, 128], BF16, tag="idm")
                nc.sync.dma_start(idm[:], id_d[:])
                wkvT = [
                    pp.tile([128, 2 * DIM], F32R, name=f"wkvT{t}", tag=f"wkvT{t}")
                    for t in range(4)
                ]
                for t in range(4):
                    nc.sync.dma_start(wkvT[t][:], wkvT_d[t * 128 : (t + 1) * 128, :])
                wout = [
                    pp.tile([64, DIM], F32R, name=f"wout{h}", tag=f"wout{h}")
                    for h in range(H)
                ]
                for h in range(H):
                    nc.sync.dma_start(wout[h][:], wout_d[h * 64 : (h + 1) * 64, :])
                bout = pp.tile([1, DIM], F32R, tag="bout")
                nc.sync.dma_start(bout[:], bout_d[:])

                # c = scale * sum(w_pos) on every partition; wposr /= c so the
                # pos-bias accumulates pre-divided and exp(scale=c) restores it.
                c_ap = pp.tile([128, 1], F32, tag="c_ap")
                ic_ap = pp.tile([128, 1], F32, tag="ic_ap")
                nc.vector.tensor_reduce(c_ap[:], wposr[:, 0, :], axis=AX.X, op=ALU.add)
                nc.scalar.mul(c_ap[:], c_ap[:], SCALE)
                nc.vector.reciprocal(ic_ap[:], c_ap[:])
                with nc.allow_low_precision(reason="w_pos/c in bf16 is intended"):
                    nc.vector.tensor_scalar_mul(wposr[:], wposr[:], ic_ap[:])

                # posT_sb[jt] = pos_bias^T / c for key-tile jt (128 j x 256 i)
                posT_sb = [
                    pp.tile([128, IQ], BF16, name=f"posT{j}", tag=f"posT{j}")
                    for j in range(JT)
                ]
                def emit_pos():
                    # ---- pos-bias phase: all j, this core's i-quarter ----

                    for jt in range(JT):
                        for ic in range(IQ // POS_CHUNK):
                            sl = slice(ic * POS_CHUNK, (ic + 1) * POS_CHUNK)
                            pt = pos_pool.tile(
                                [128, POS_CHUNK, POS_DIM], BF16, name="pchunk", tag="pchunk"
                            )
                            nc.sync.dma_start(
                                pt[:], posT_d[jt * 128 : (jt + 1) * 128, sl, :]
                            )
                            nc.vector.tensor_tensor(pt[:], pt[:], wposr[:], op=ALU.mult)
                            with nc.allow_low_precision(
                                reason="pos bias flows in bf16 by design"
                            ):
                                nc.vector.tensor_reduce(
                                    posT_sb[jt][:, sl], pt[:], axis=AX.X, op=ALU.add
                                )


                rzb = pp.tile([64, IQ], F32, tag="rzb")
                rzrow = pp.tile([65, IQ], F32, tag="rzrow")
                rz_bounce = dram.tile([1, IQ], F32)

                # ---- per batch: kv, attention, projection ----
                UT_sets = {
                    s2: [
                        pp.tile([64, IQ], F32R, name=f"UT{h}_{s2}", tag=f"UT{h}_{s2}")
                        for h in range(H)
                    ]
                    for s2 in (0, 1)
                }
                kv_tiles = {}

                def emit_kv(b):
                    s2 = b % 2  # double-buffer set for cross-batch overlap
                    xT = [
                        pp.tile([128, N], F32R, name=f"xT{t}_{s2}", tag=f"xT{t}_{s2}")
                        for t in range(4)
                    ]
                    for t in range(4):
                        nc.sync.dma_start(
                            xT[t][:], xT_d[b, t * 128 : (t + 1) * 128, :]
                        )
                    xqT = [
                        pp.tile(
                            [128, IQ], F32R, name=f"xqT{t}_{s2}", tag=f"xqT{t}_{s2}"
                        )
                        for t in range(4)
                    ]
                    for t in range(4):
                        nc.sync.dma_start(
                            xqT[t][:], xqT_d[b, t * 128 : (t + 1) * 128, :]
                        )

                    kT = [
                        pp.tile([128, N], F32R, name=f"kT{t}_{s2}", tag=f"kT{t}_{s2}")
                        for t in range(4)
                    ]
                    for t in range(4):
                        for nchunk in range(2):
                            ps = mmps.tile([128, 512], F32, name="mmtile", tag="mm")
                            for dc in range(4):
                                nc.tensor.matmul(
                                    ps[:],
                                    wkvT[dc][:, t * 128 : (t + 1) * 128],
                                    xT[dc][:, nchunk * 512 : (nchunk + 1) * 512],
                                    start=(dc == 0),
                                    stop=(dc == 3),
                                )
                            nc.vector.tensor_copy(
                                kT[t][:, nchunk * 512 : (nchunk + 1) * 512], ps[:]
                            )
                    kQT = [
                        pp.tile(
                            [128, IQ], F32R, name=f"kQT{t}_{s2}", tag=f"kQT{t}_{s2}"
                        )
                        for t in range(4)
                    ]
                    for t in range(4):
                        ps = mmps.tile([128, IQ], F32, name="mmq", tag="mm")
                        for dc in range(4):
                            nc.tensor.matmul(
                                ps[:],
                                wkvT[dc][:, t * 128 : (t + 1) * 128],
                                xqT[dc][:],
                                start=(dc == 0),
                                stop=(dc == 3),
                            )
                        nc.vector.tensor_copy(kQT[t][:], ps[:])

                    vext = [
                        pp.tile(
                            [128, H, D + 1],
                            F32R,
                            name=f"vext{t}_{s2}",
                            tag=f"vext{t}_{s2}",
                        )
                        for t in range(JT)
                    ]
                    for nt in range(JT):
                        ps = mmps.tile([128, 512], F32, name="mmtile", tag="mm")
                        for dc in range(4):
                            nc.tensor.matmul(
                                ps[:],
                                xT[dc][:, nt * 128 : (nt + 1) * 128],
                                wkvT[dc][:, DIM : 2 * DIM],
                                start=(dc == 0),
                                stop=(dc == 3),
                            )
                        nc.sync.dma_start(vext[nt][:, :, D : D + 1], ones_d[:, 0:H])
                        nc.vector.tensor_copy(
                            vext[nt][:, :, 0:D],
                            ps[:].rearrange("p (h d) -> p h d", h=H),
                        )
                    kv_tiles[b] = (kT, kQT, vext)

                def emit_attn(b):
                    s2 = b % 2
                    kT, kQT, vext = kv_tiles[b]
                    UT = UT_sets[s2]
                    for h in range(H):
                        kt = kT[h // 2]
                        kq = kQT[h // 2]
                        pr = slice(64 * (h % 2), 64 * (h % 2) + 64)
                        up = upps.tile([D + 1, IQ], F32, name="uptile", tag="up")
                        for jg in range(JT // 4):  # groups of 4 key-tiles
                            dots = dotsps.tile(
                                [128, 4 * IQ], F32, name="dotstile", tag="dots"
                            )
                            for q in range(4):
                                jt = jg * 4 + q
                                qsl = slice(q * IQ, (q + 1) * IQ)
                                nc.tensor.matmul(
                                    dots[:, qsl],
                                    kt[pr, jt * 128 : (jt + 1) * 128],
                                    kq[pr, :],
                                    start=True,
                                    stop=False,
                                )
                                nc.tensor.matmul(
                                    dots[:, qsl],
                                    idm[:],
                                    posT_sb[jt][:],
                                    start=False,
                                    stop=True,
                                )
                            es = epool.tile(
                                [128, 4 * IQ], F32R, name="expS", tag="expS"
                            )
                            nc.scalar.activation(
                                es[:], dots[:], ACTF.Exp, scale=c_ap[:]
                            )
                            for q in range(4):
                                jt = jg * 4 + q
                                qsl = slice(q * IQ, (q + 1) * IQ)
                                nc.tensor.matmul(
                                    up[:],
                                    vext[jt][:, h, :],
                                    es[:, qsl],
                                    start=(jt == 0),
                                    stop=(jt == JT - 1),
                                )
                        # row 64 of up = Z; normalize U while copying out
                        nc.vector.reciprocal(rzrow[64:65, :], up[64:65, :])
                        nc.sync.dma_start(rz_bounce[:], rzrow[64:65, :])
                        nc.sync.dma_start(
                            rzb[:], rz_bounce[:].to_broadcast([64, IQ])
                        )
                        nc.vector.tensor_tensor(
                            UT[h][:], up[0:64, :], rzb[:], op=ALU.mult
                        )

                def emit_final(b):
                    s2 = b % 2
                    UT = UT_sets[s2]
                    for it in range(IQ // 128):
                        isl = slice(it * 128, (it + 1) * 128)
                        fps = mmps.tile([128, 512], F32, name="mmtile", tag="mm")
                        for h in range(H):
                            nc.tensor.matmul(
                                fps[:],
                                UT[h][:, isl],
                                wout[h][:],
                                start=(h == 0),
                                stop=False,
                            )
                        nc.tensor.matmul(
                            fps[:], ones1[:], bout[:], start=False, stop=True
                        )
                        ot = opool.tile([128, 512], F32, name="osb", tag="osb")
                        nc.vector.tensor_copy(ot[:], fps[:])
                        nc.sync.dma_start(y_d[b, isl, :], ot[:])

                emit_kv(0)
                emit_kv(1)
                emit_pos()
                emit_attn(0)
                emit_final(0)
                emit_kv(2)
                emit_attn(1)
                emit_final(1)
                emit_kv(3)
                emit_attn(2)
                emit_final(2)
                emit_attn(3)
                emit_final(3)

    nc.compile()
    return nc


_CACHE = {}


def _get_program():
    if "nc" not in _CACHE:
        _CACHE["nc"] = build_program()
    return _CACHE["nc"]


def _host_shard(x, pos, W_kv, W_out, b_out, w_pos, b_pos):
    """Build the 8 per-core input maps (pure layout work, no math)."""
    import ml_dtypes

    x = np.asarray(x, dtype=np.float32)
    pos = np.asarray(pos, dtype=np.float32)
    W_kv = np.asarray(W_kv, dtype=np.float32)
    W_out = np.asarray(W_out, dtype=np.float32)
    b_out = np.asarray(b_out, dtype=np.float32)
    w_pos = np.asarray(w_pos, dtype=np.float32)

    wkvT = np.ascontiguousarray(W_kv.T)  # (512, 1024)
    wout = np.ascontiguousarray(W_out.T)  # (512, 512)
    boutr = b_out.reshape(1, DIM)
    wposr = np.ascontiguousarray(
        np.broadcast_to(w_pos.astype(ml_dtypes.bfloat16), (128, POS_CHUNK, POS_DIM))
    )
    ones_arr = np.ones((128, 128), dtype=np.float32)
    id_arr = np.eye(128, dtype=ml_dtypes.bfloat16)

    in_maps = []
    for c in range(NC):
        g, s = c // 4, c % 4
        bs = slice(4 * g, 4 * g + BPC)
        isl = slice(s * IQ, (s + 1) * IQ)
        xT = np.ascontiguousarray(x[bs].transpose(0, 2, 1))  # (4, 512, 1024)
        xqT = np.ascontiguousarray(x[bs, isl].transpose(0, 2, 1))  # (4, 512, 256)
        posT = np.ascontiguousarray(
            pos[0, isl, :, :].transpose(1, 0, 2).astype(ml_dtypes.bfloat16)
        )  # (1024 j, 256 i, 50) bf16
        in_maps.append(
            {
                "xT": xT,
                "xqT": xqT,
                "wkvT": wkvT,
                "wout": wout,
                "bout": boutr,
                "wposr": wposr,
                "posT": posT,
                "ones": ones_arr,
                "idm": id_arr,
            }
        )
    return in_maps


def kernel(**inputs) -> np.ndarray:
    nc = _get_program()
    in_maps = _host_shard(**inputs)
    res = run_bass_kernel_spmd(nc, in_maps, list(range(NC)))
    out = np.empty((B, N, DIM), dtype=np.float32)
    for c in range(NC):
        g, s = c // 4, c % 4
        out[4 * g : 4 * g + BPC, s * IQ : (s + 1) * IQ, :] = res.results[c]["y"]
    return out


if __name__ == "__main__":
    import reference

    inputs = {k: np.asarray(v) for k, v in reference.setup_inputs().items()}
    expected = np.asarray(reference.reference(**inputs))
    actual = kernel(**inputs)
    err = np.abs(actual - expected).max()
    rel = err / np.abs(expected).max()
    print(f"absmax err: {err:.3e}  rel: {rel:.3e}")



# revision 2
# speedup vs baseline: 1.4439x; 1.4439x over previous
"""Trainium2 Bass kernel for nn_Attention_KV (dense transformer attention
with K=Q sharing and a linear positional bias), distributed over 8 cores.

Sharding: 2 batch-groups x 4 query-quarters (collective-free). Core
c = 4*g + s owns batches 4g..4g+3 and query rows i in [256*s, 256*(s+1)).
The positional bias pos_bias(i,j) is head/batch independent but sharded
by i-quarter, so each core loads exactly the pos slice it consumes.

Layout tricks (all pure host-side layout, no host math):
  - The token (j) axis is ROLLED per core so the core's own query
    quarter occupies columns 0:IQ of k^T. The query block is then just a
    slice of kT - no separate x_q input, no separate k_q matmuls - while
    keeping the program identical across cores (SPMD). j only ever
    appears inside sums, so the roll is invisible in the output.
  - pos is pre-transposed to [jt, j, i, p] blocks so each j-tile loads
    as one fully contiguous 3.2MB HBM stream.

All attention math keeps scores TRANSPOSED (keys j on partitions,
queries i on the free axis); dots = k @ k^T is symmetric so this is
free, and softmax + the attn @ v contraction need no on-chip transpose:
  - scores^T = c*dots lands in PSUM (c = scale*sum(w_pos))
  - es = Exp(scale=c)(dots) * P where P = exp(pos_bias^T) is computed
    ONCE per core (instead of re-adding pos via identity matmuls for
    every batch*head)
  - attn@v as lhsT = v_ext (ones column appended -> row 64 of the
    result is the softmax denominator Z), rhs = es
  - normalization folded into the PSUM->SBUF copy of U; 1/Z is
    broadcast across partitions by the (otherwise idle) GpSimd engine
Everything flows in bf16 (f32 PSUM accumulation); softmax averaging
damps the rounding error far below the 2e-2 gate. b_pos (a scalar added
to every score) is dropped: softmax is shift invariant.
"""

import sys

sys.path.insert(0, "/opt/trn_rl_repo")

import numpy as np

import concourse.bacc as bacc
import concourse.bass as bass
import concourse.mybir as mybir
from concourse import tile
from concourse.bass_utils import run_bass_kernel_spmd

B, N, DIM, H, POS_DIM = 8, 1024, 512, 8, 50
D = DIM // H  # 64
NC = 8  # cores
BPC = 4  # batches per core
IQ = 256  # query rows per core
JT = N // 128  # 8 j-tiles
SCALE = float(DIM) ** -0.5

F32 = mybir.dt.float32
F32R = mybir.dt.float32r
BF16 = mybir.dt.bfloat16
AX = mybir.AxisListType
ALU = mybir.AluOpType
ACTF = mybir.ActivationFunctionType

POS_CHUNK = 64  # i-columns of pos processed per DVE reduce


def build_program(reps: int = 1):
    nc = bacc.Bacc("TRN2", target_bir_lowering=False, debug=False)

    # ---- DRAM parameters (per-core) ----
    xT_d = nc.declare_dram_parameter("xT", [BPC, DIM, N], BF16, isOutput=False)
    wkvT_d = nc.declare_dram_parameter("wkvT", [DIM, 2 * DIM], BF16, isOutput=False)
    wout_d = nc.declare_dram_parameter("wout", [DIM, DIM], BF16, isOutput=False)
    bout_d = nc.declare_dram_parameter("bout", [1, DIM], BF16, isOutput=False)
    wposr_d = nc.declare_dram_parameter(
        "wposr", [128, POS_CHUNK, POS_DIM], BF16, isOutput=False
    )
    posT_d = nc.declare_dram_parameter(
        "posT", [JT, 128, IQ, POS_DIM], BF16, isOutput=False
    )
    y_d = nc.declare_dram_parameter("y", [BPC, IQ, DIM], F32, isOutput=True)

    with tile.TileContext(nc) as tc:
        with (
            tc.tile_pool(name="persist", bufs=1) as pp,
            tc.tile_pool(name="pos_in", bufs=2) as pos_pool,
            tc.tile_pool(name="exps", bufs=3) as epool,
            tc.tile_pool(name="rzs", bufs=2) as rzpool,
            tc.tile_pool(name="outsb", bufs=2) as opool,
            tc.tile_pool(name="mm_ps", bufs=2, space="PSUM") as mmps,
            tc.tile_pool(name="dots_ps", bufs=2, space="PSUM") as dotsps,
            tc.tile_pool(name="up_ps", bufs=2, space="PSUM") as upps,
        ):
            for _rep in range(reps):
                # ---- preload small tensors + weights ----
                wposr = pp.tile([128, POS_CHUNK, POS_DIM], BF16, tag="wposr")
                nc.sync.dma_start(wposr[:], wposr_d[:])
                wkvT = [
                    pp.tile([128, 2 * DIM], BF16, name=f"wkvT{t}", tag=f"wkvT{t}")
                    for t in range(4)
                ]
                for t in range(4):
                    nc.sync.dma_start(wkvT[t][:], wkvT_d[t * 128 : (t + 1) * 128, :])
                wout = [
                    pp.tile([64, DIM], BF16, name=f"wout{h}", tag=f"wout{h}")
                    for h in range(H)
                ]
                for h in range(H):
                    nc.sync.dma_start(wout[h][:], wout_d[h * 64 : (h + 1) * 64, :])
                bout = pp.tile([1, DIM], BF16, tag="bout")
                nc.sync.dma_start(bout[:], bout_d[:])
                ones1 = pp.tile([1, 128], BF16, tag="ones1")
                nc.vector.memset(ones1[:], 1.0)

                # c = scale * sum(w_pos) on every partition (exp scale)
                c_ap = pp.tile([128, 1], F32, tag="c_ap")
                nc.vector.tensor_reduce(c_ap[:], wposr[:, 0, :], axis=AX.X, op=ALU.add)
                nc.scalar.mul(c_ap[:], c_ap[:], SCALE)

                # v_ext tiles (both double-buffer sets), ones column set once
                vext_sets = {
                    s2: [
                        pp.tile(
                            [128, H, D + 1],
                            BF16,
                            name=f"vext{t}_{s2}",
                            tag=f"vext{t}_{s2}",
                        )
                        for t in range(JT)
                    ]
                    for s2 in (0, 1)
                }
                for s2 in (0, 1):
                    for t in range(JT):
                        nc.vector.memset(vext_sets[s2][t][:, :, D : D + 1], 1.0)

                # P = exp(pos_bias^T) for this core's i-quarter, all j
                pbias = pp.tile([128, JT, IQ], BF16, tag="pbias")
                pexp = pp.tile([128, JT, IQ], BF16, tag="pexp")

                def emit_pos():
                    for jt in range(JT):
                        pt = pos_pool.tile(
                            [128, IQ, POS_DIM], BF16, name="pchunk", tag="pchunk"
                        )
                        nc.sync.dma_start(pt[:], posT_d[jt])
                        for ic in range(IQ // POS_CHUNK):
                            sl = slice(ic * POS_CHUNK, (ic + 1) * POS_CHUNK)
                            nc.vector.tensor_tensor(
                                pt[:, sl, :], pt[:, sl, :], wposr[:], op=ALU.mult
                            )
                            with nc.allow_low_precision(
                                reason="pos bias flows in bf16 by design"
                            ):
                                nc.vector.tensor_reduce(
                                    pbias[:, jt, sl], pt[:, sl, :], axis=AX.X, op=ALU.add
                                )
                        nc.scalar.activation(
                            pexp[:, jt, :], pbias[:, jt, :], ACTF.Exp
                        )

                # ---- per batch: kv, attention, projection ----
                UT_sets = {
                    s2: [
                        pp.tile([64, IQ], BF16, name=f"UT{h}_{s2}", tag=f"UT{h}_{s2}")
                        for h in range(H)
                    ]
                    for s2 in (0, 1)
                }
                kv_tiles = {}

                def emit_kv(b):
                    s2 = b % 2  # double-buffer set for cross-batch overlap
                    xT = [
                        pp.tile([128, N], BF16, name=f"xT{t}_{s2}", tag=f"xT{t}_{s2}")
                        for t in range(4)
                    ]
                    for t in range(4):
                        nc.sync.dma_start(
                            xT[t][:], xT_d[b, t * 128 : (t + 1) * 128, :]
                        )

                    kT = [
                        pp.tile([128, N], BF16, name=f"kT{t}_{s2}", tag=f"kT{t}_{s2}")
                        for t in range(4)
                    ]
                    for t in range(4):
                        for nchunk in range(2):
                            ps = mmps.tile([128, 512], F32, name="mmtile", tag="mm")
                            for dc in range(4):
                                nc.tensor.matmul(
                                    ps[:],
                                    wkvT[dc][:, t * 128 : (t + 1) * 128],
                                    xT[dc][:, nchunk * 512 : (nchunk + 1) * 512],
                                    start=(dc == 0),
                                    stop=(dc == 3),
                                )
                            nc.vector.tensor_copy(
                                kT[t][:, nchunk * 512 : (nchunk + 1) * 512], ps[:]
                            )

                    vext = vext_sets[s2]
                    for nt in range(JT):
                        ps = mmps.tile([128, 512], F32, name="mmtile", tag="mm")
                        for dc in range(4):
                            nc.tensor.matmul(
                                ps[:],
                                xT[dc][:, nt * 128 : (nt + 1) * 128],
                                wkvT[dc][:, DIM : 2 * DIM],
                                start=(dc == 0),
                                stop=(dc == 3),
                            )
                        nc.vector.tensor_copy(
                            vext[nt][:, :, 0:D],
                            ps[:].rearrange("p (h d) -> p h d", h=H),
                        )
                    kv_tiles[b] = (kT, vext)

                def emit_attn(b):
                    s2 = b % 2
                    kT, vext = kv_tiles[b]
                    UT = UT_sets[s2]
                    for h in range(H):
                        kt = kT[h // 2]
                        pr = slice(64 * (h % 2), 64 * (h % 2) + 64)
                        up = upps.tile([D + 1, IQ], F32, name="uptile", tag="up")
                        for jg in range(JT // 4):  # groups of 4 key-tiles
                            dots = dotsps.tile(
                                [128, 4 * IQ], F32, name="dotstile", tag="dots"
                            )
                            for q in range(4):
                                jt = jg * 4 + q
                                qsl = slice(q * IQ, (q + 1) * IQ)
                                nc.tensor.matmul(
                                    dots[:, qsl],
                                    kt[pr, jt * 128 : (jt + 1) * 128],
                                    kt[pr, 0:IQ],
                                    start=True,
                                    stop=True,
                                )
                            es = epool.tile(
                                [128, 4 * IQ], BF16, name="expS", tag="expS"
                            )
                            nc.scalar.activation(
                                es[:], dots[:], ACTF.Exp, scale=c_ap[:]
                            )
                            nc.vector.tensor_tensor(
                                es[:],
                                es[:],
                                pexp[:, jg * 4 : (jg + 1) * 4, :].rearrange(
                                    "p a b -> p (a b)"
                                ),
                                op=ALU.mult,
                            )
                            for q in range(4):
                                jt = jg * 4 + q
                                qsl = slice(q * IQ, (q + 1) * IQ)
                                nc.tensor.matmul(
                                    up[:],
                                    vext[jt][:, h, :],
                                    es[:, qsl],
                                    start=(jt == 0),
                                    stop=(jt == JT - 1),
                                )
                        # row 64 of up = Z; normalize U while copying out
                        rz = rzpool.tile([1, IQ], F32, name="rz", tag="rz")
                        nc.vector.reciprocal(rz[:], up[64:65, :])
                        rzb = rzpool.tile([64, IQ], F32, name="rzb", tag="rzb")
                        nc.gpsimd.partition_broadcast(rzb[:], rz[:])
                        nc.vector.tensor_tensor(
                            UT[h][:], up[0:64, :], rzb[:], op=ALU.mult
                        )

                def emit_final(b):
                    s2 = b % 2
                    UT = UT_sets[s2]
                    for it in range(IQ // 128):
                        isl = slice(it * 128, (it + 1) * 128)
                        fps = mmps.tile([128, 512], F32, name="mmtile", tag="mm")
                        for h in range(H):
                            nc.tensor.matmul(
                                fps[:],
                                UT[h][:, isl],
                                wout[h][:],
                                start=(h == 0),
                                stop=False,
                            )
                        nc.tensor.matmul(
                            fps[:], ones1[:], bout[:], start=False, stop=True
                        )
                        ot = opool.tile([128, 512], F32, name="osb", tag="osb")
                        nc.vector.tensor_copy(ot[:], fps[:])
                        nc.sync.dma_start(y_d[b, isl, :], ot[:])

                emit_kv(0)
                emit_pos()
                emit_kv(1)
                emit_attn(0)
                emit_final(0)
                emit_kv(2)
                emit_attn(1)
                emit_final(1)
                emit_kv(3)
                emit_attn(2)
                emit_final(2)
                emit_attn(3)
                emit_final(3)

    nc.compile()
    return nc


_CACHE = {}


def _get_program():
    if "nc" not in _CACHE:
        _CACHE["nc"] = build_program()
    return _CACHE["nc"]


def _host_shard(x, pos, W_kv, W_out, b_out, w_pos, b_pos):
    """Build the 8 per-core input maps (pure layout work, no math)."""
    import ml_dtypes

    bf16 = ml_dtypes.bfloat16
    x = np.asarray(x, dtype=np.float32)
    pos = np.asarray(pos, dtype=np.float32)
    W_kv = np.asarray(W_kv, dtype=np.float32)
    W_out = np.asarray(W_out, dtype=np.float32)
    b_out = np.asarray(b_out, dtype=np.float32)
    w_pos = np.asarray(w_pos, dtype=np.float32)

    wkvT = np.ascontiguousarray(W_kv.T.astype(bf16))  # (512, 1024)
    wout = np.ascontiguousarray(W_out.T.astype(bf16))  # (512, 512)
    boutr = np.ascontiguousarray(b_out.reshape(1, DIM).astype(bf16))
    wposr = np.ascontiguousarray(
        np.broadcast_to(w_pos.astype(bf16), (128, POS_CHUNK, POS_DIM))
    )

    in_maps = []
    for c in range(NC):
        g, s = c // 4, c % 4
        bs = slice(4 * g, 4 * g + BPC)
        isl = slice(s * IQ, (s + 1) * IQ)
        # roll tokens so this core's query quarter is columns 0:IQ
        xr = np.roll(x[bs], -s * IQ, axis=1)  # (4, 1024, 512)
        xT = np.ascontiguousarray(xr.transpose(0, 2, 1).astype(bf16))  # (4,512,1024)
        posr = np.roll(pos[0, isl, :, :], -s * IQ, axis=1)  # (256 i, 1024 j, 50)
        posT = np.ascontiguousarray(
            posr.transpose(1, 0, 2).reshape(JT, 128, IQ, POS_DIM).astype(bf16)
        )
        in_maps.append(
            {
                "xT": xT,
                "wkvT": wkvT,
                "wout": wout,
                "bout": boutr,
                "wposr": wposr,
                "posT": posT,
            }
        )
    return in_maps


def kernel(**inputs) -> np.ndarray:
    nc = _get_program()
    in_maps = _host_shard(**inputs)
    res = run_bass_kernel_spmd(nc, in_maps, list(range(NC)))
    out = np.empty((B, N, DIM), dtype=np.float32)
    for c in range(NC):
        g, s = c // 4, c % 4
        out[4 * g : 4 * g + BPC, s * IQ : (s + 1) * IQ, :] = res.results[c]["y"]
    return out


if __name__ == "__main__":
    import reference

    inputs = {k: np.asarray(v) for k, v in reference.setup_inputs().items()}
    expected = np.asarray(reference.reference(**inputs))
    actual = kernel(**inputs)
    err = np.abs(actual - expected).max()
    rel = err / np.abs(expected).max()
    print(f"absmax err: {err:.3e}  rel: {rel:.3e}")


# revision 13
# speedup vs baseline: 1.4663x; 1.0155x over previous
"""Trainium2 Bass kernel for nn_Attention_KV (dense transformer attention
with K=Q sharing and a linear positional bias), distributed over 8 cores.

Sharding: 2 batch-groups x 4 query-quarters (collective-free). Core
c = 4*g + s owns batches 4g..4g+3 and query rows i in [256*s, 256*(s+1)).
The positional bias pos_bias(i,j) is head/batch independent but sharded
by i-quarter, so each core loads exactly the pos slice it consumes.

Layout tricks (all pure host-side layout, no host math):
  - The token (j) axis is ROLLED per core so the core's own query
    quarter occupies columns 0:IQ of k^T. The query block is then just a
    slice of kT - no separate x_q input, no separate k_q matmuls - while
    keeping the program identical across cores (SPMD). j only ever
    appears inside sums, so the roll is invisible in the output.
  - pos is pre-transposed to [jt, j, i, p] blocks so each j-tile loads
    as one fully contiguous 3.2MB HBM stream.

All attention math keeps scores TRANSPOSED (keys j on partitions,
queries i on the free axis); dots = k @ k^T is symmetric so this is
free, and softmax + the attn @ v contraction need no on-chip transpose:
  - scores^T = c*dots lands in PSUM (c = scale*sum(w_pos))
  - es = Exp(scale=c)(dots) * P where P = exp(pos_bias^T) is computed
    ONCE per core (instead of re-adding pos via identity matmuls for
    every batch*head)
  - attn@v as lhsT = v_ext (ones column appended -> row 64 of the
    result is the softmax denominator Z), rhs = es
  - normalization folded into the PSUM->SBUF copy of U; 1/Z is
    broadcast across partitions by the (otherwise idle) GpSimd engine
Everything flows in bf16 (f32 PSUM accumulation); softmax averaging
damps the rounding error far below the 2e-2 gate. b_pos (a scalar added
to every score) is dropped: softmax is shift invariant.
"""

import sys

sys.path.insert(0, "/opt/trn_rl_repo")

import numpy as np

import concourse.bacc as bacc
import concourse.bass as bass
import concourse.mybir as mybir
from concourse import tile
from concourse.bass_utils import run_bass_kernel_spmd

B, N, DIM, H, POS_DIM = 8, 1024, 512, 8, 50
D = DIM // H  # 64
NC = 8  # cores
BPC = 4  # batches per core
IQ = 256  # query rows per core
JT = N // 128  # 8 j-tiles
SCALE = float(DIM) ** -0.5

F32 = mybir.dt.float32
F32R = mybir.dt.float32r
BF16 = mybir.dt.bfloat16
AX = mybir.AxisListType
ALU = mybir.AluOpType
ACTF = mybir.ActivationFunctionType

POS_CHUNK = 64  # i-columns of pos processed per DVE reduce


def build_program(reps: int = 1):
    nc = bacc.Bacc("TRN2", target_bir_lowering=False, debug=False)

    # ---- DRAM parameters (per-core) ----
    xT_d = nc.declare_dram_parameter("xT", [BPC, DIM, N], BF16, isOutput=False)
    wkvT_d = nc.declare_dram_parameter("wkvT", [DIM, 2 * DIM], BF16, isOutput=False)
    wout_d = nc.declare_dram_parameter("wout", [DIM, DIM], BF16, isOutput=False)
    bout_d = nc.declare_dram_parameter("bout", [1, DIM], BF16, isOutput=False)
    wposr_d = nc.declare_dram_parameter(
        "wposr", [128, IQ, POS_DIM], BF16, isOutput=False
    )
    posT_d = nc.declare_dram_parameter(
        "posT", [JT, 128, IQ, POS_DIM], BF16, isOutput=False
    )
    y_d = nc.declare_dram_parameter("y", [BPC, IQ, DIM], F32, isOutput=True)

    with tile.TileContext(nc) as tc:
        with (
            tc.tile_pool(name="persist", bufs=1) as pp,
            tc.tile_pool(name="pos_in", bufs=2) as pos_pool,
            tc.tile_pool(name="exps", bufs=4) as epool,
            tc.tile_pool(name="rzs", bufs=2) as rzpool,
            tc.tile_pool(name="outsb", bufs=2) as opool,
            tc.tile_pool(name="mm_ps", bufs=2, space="PSUM") as mmps,
            tc.tile_pool(name="dots_ps", bufs=2, space="PSUM") as dotsps,
            tc.tile_pool(name="up_ps", bufs=2, space="PSUM") as upps,
        ):
            for _rep in range(reps):
                # ---- preload small tensors + weights ----
                wposr = pp.tile([128, IQ, POS_DIM], BF16, tag="wposr")
                nc.sync.dma_start(wposr[:], wposr_d[:])
                wkvT = [
                    pp.tile([128, 2 * DIM], BF16, name=f"wkvT{t}", tag=f"wkvT{t}")
                    for t in range(4)
                ]
                for t in range(4):
                    nc.sync.dma_start(wkvT[t][:], wkvT_d[t * 128 : (t + 1) * 128, :])
                wout = [
                    pp.tile([64, DIM], BF16, name=f"wout{h}", tag=f"wout{h}")
                    for h in range(H)
                ]
                for h in range(H):
                    nc.sync.dma_start(wout[h][:], wout_d[h * 64 : (h + 1) * 64, :])
                bout = pp.tile([1, DIM], BF16, tag="bout")
                nc.sync.dma_start(bout[:], bout_d[:])
                ones1 = pp.tile([1, 128], BF16, tag="ones1")
                nc.vector.memset(ones1[:], 1.0)

                # c = scale * sum(w_pos) on every partition (exp scale)
                c_ap = pp.tile([128, 1], F32, tag="c_ap")
                nc.vector.tensor_reduce(c_ap[:], wposr[:, 0, :], axis=AX.X, op=ALU.add)
                nc.scalar.mul(c_ap[:], c_ap[:], SCALE)

                # v_ext tiles (both double-buffer sets), ones column set once
                vext_sets = {
                    s2: [
                        pp.tile(
                            [128, H, D + 1],
                            BF16,
                            name=f"vext{t}_{s2}",
                            tag=f"vext{t}_{s2}",
                        )
                        for t in range(JT)
                    ]
                    for s2 in (0, 1)
                }
                for s2 in (0, 1):
                    for t in range(JT):
                        nc.vector.memset(vext_sets[s2][t][:, :, D : D + 1], 1.0)

                # P = exp(pos_bias^T) for this core's i-quarter, all j
                pbias = pp.tile([128, JT, IQ], BF16, tag="pbias")
                pexp = pp.tile([128, JT, IQ], BF16, tag="pexp")

                def emit_pos():
                    for jt in range(JT):
                        pt = pos_pool.tile(
                            [128, IQ, POS_DIM], BF16, name="pchunk", tag="pchunk"
                        )
                        nc.sync.dma_start(pt[:], posT_d[jt])
                        nc.vector.tensor_tensor(
                            pt[:].rearrange("p a b -> p (a b)"),
                            pt[:].rearrange("p a b -> p (a b)"),
                            wposr[:].rearrange("p a b -> p (a b)"),
                            op=ALU.mult,
                        )
                        with nc.allow_low_precision(
                            reason="pos bias flows in bf16 by design"
                        ):
                            nc.vector.tensor_reduce(
                                pbias[:, jt, :], pt[:], axis=AX.X, op=ALU.add
                            )
                        nc.scalar.activation(
                            pexp[:, jt, :], pbias[:, jt, :], ACTF.Exp
                        )

                # ---- per batch: kv, attention, projection ----
                UT_sets = {
                    s2: [
                        pp.tile([64, IQ], BF16, name=f"UT{h}_{s2}", tag=f"UT{h}_{s2}")
                        for h in range(H)
                    ]
                    for s2 in (0, 1)
                }
                kv_tiles = {}

                def emit_kv(b):
                    s2 = b % 2  # double-buffer set for cross-batch overlap
                    xT = [
                        pp.tile([128, N], BF16, name=f"xT{t}_{s2}", tag=f"xT{t}_{s2}")
                        for t in range(4)
                    ]
                    for t in range(4):
                        nc.sync.dma_start(
                            xT[t][:], xT_d[b, t * 128 : (t + 1) * 128, :]
                        )

                    kT = [
                        pp.tile([128, N], BF16, name=f"kT{t}_{s2}", tag=f"kT{t}_{s2}")
                        for t in range(4)
                    ]
                    for t in range(4):
                        # both n-halves interleaved so consecutive matmuls
                        # share the same stationary weights
                        pss = [
                            mmps.tile([128, 512], F32, name="mmtile", tag="mm")
                            for _ in range(2)
                        ]
                        for dc in range(4):
                            for nchunk in range(2):
                                nc.tensor.matmul(
                                    pss[nchunk][:],
                                    wkvT[dc][:, t * 128 : (t + 1) * 128],
                                    xT[dc][:, nchunk * 512 : (nchunk + 1) * 512],
                                    start=(dc == 0),
                                    stop=(dc == 3),
                                )
                        for nchunk in range(2):
                            nc.vector.tensor_copy(
                                kT[t][:, nchunk * 512 : (nchunk + 1) * 512],
                                pss[nchunk][:],
                            )

                    vext = vext_sets[s2]
                    for nt in range(JT):
                        ps = mmps.tile([128, 512], F32, name="mmtile", tag="mm")
                        for dc in range(4):
                            nc.tensor.matmul(
                                ps[:],
                                xT[dc][:, nt * 128 : (nt + 1) * 128],
                                wkvT[dc][:, DIM : 2 * DIM],
                                start=(dc == 0),
                                stop=(dc == 3),
                            )
                        nc.scalar.copy(
                            vext[nt][:, :, 0:D],
                            ps[:].rearrange("p (h d) -> p h d", h=H),
                        )
                    kv_tiles[b] = (kT, vext)

                def emit_attn(b):
                    s2 = b % 2
                    kT, vext = kv_tiles[b]
                    UT = UT_sets[s2]
                    for h in range(H):
                        kt = kT[h // 2]
                        pr = slice(64 * (h % 2), 64 * (h % 2) + 64)
                        up = upps.tile([D + 1, IQ], F32, name="uptile", tag="up")
                        for jg in range(JT // 4):  # groups of 4 key-tiles
                            dots = dotsps.tile(
                                [128, 4 * IQ], F32, name="dotstile", tag="dots"
                            )
                            for q in range(4):
                                jt = jg * 4 + q
                                qsl = slice(q * IQ, (q + 1) * IQ)
                                nc.tensor.matmul(
                                    dots[:, qsl],
                                    kt[pr, jt * 128 : (jt + 1) * 128],
                                    kt[pr, 0:IQ],
                                    start=True,
                                    stop=True,
                                )
                            es = epool.tile(
                                [128, 4 * IQ], BF16, name="expS", tag="expS"
                            )
                            nc.scalar.activation(
                                es[:], dots[:], ACTF.Exp, scale=c_ap[:]
                            )
                            nc.vector.tensor_tensor(
                                es[:],
                                es[:],
                                pexp[:, jg * 4 : (jg + 1) * 4, :].rearrange(
                                    "p a b -> p (a b)"
                                ),
                                op=ALU.mult,
                            )
                            for q in range(4):
                                jt = jg * 4 + q
                                qsl = slice(q * IQ, (q + 1) * IQ)
                                nc.tensor.matmul(
                                    up[:],
                                    vext[jt][:, h, :],
                                    es[:, qsl],
                                    start=(jt == 0),
                                    stop=(jt == JT - 1),
                                )
                        # row 64 of up = Z; normalize U while copying out
                        rz = rzpool.tile([1, IQ], F32, name="rz", tag="rz")
                        nc.vector.reciprocal(rz[:], up[64:65, :])
                        rzb = rzpool.tile([64, IQ], F32, name="rzb", tag="rzb")
                        nc.gpsimd.partition_broadcast(rzb[:], rz[:])
                        nc.vector.tensor_tensor(
                            UT[h][:], up[0:64, :], rzb[:], op=ALU.mult
                        )

                def emit_final(b):
                    s2 = b % 2
                    UT = UT_sets[s2]
                    for it in range(IQ // 128):
                        isl = slice(it * 128, (it + 1) * 128)
                        fps = mmps.tile([128, 512], F32, name="mmtile", tag="mm")
                        for h in range(H):
                            nc.tensor.matmul(
                                fps[:],
                                UT[h][:, isl],
                                wout[h][:],
                                start=(h == 0),
                                stop=False,
                            )
                        nc.tensor.matmul(
                            fps[:], ones1[:], bout[:], start=False, stop=True
                        )
                        ot = opool.tile([128, 512], F32, name="osb", tag="osb")
                        nc.vector.tensor_copy(ot[:], fps[:])
                        nc.sync.dma_start(y_d[b, isl, :], ot[:])

                emit_kv(0)
                emit_pos()
                emit_kv(1)
                emit_attn(0)
                emit_final(0)
                emit_kv(2)
                emit_attn(1)
                emit_final(1)
                emit_kv(3)
                emit_attn(2)
                emit_final(2)
                emit_attn(3)
                emit_final(3)

    nc.compile()
    return nc


_CACHE = {}


def _get_program():
    if "nc" not in _CACHE:
        _CACHE["nc"] = build_program()
    return _CACHE["nc"]


def _host_shard(x, pos, W_kv, W_out, b_out, w_pos, b_pos):
    """Build the 8 per-core input maps (pure layout work, no math)."""
    import ml_dtypes

    bf16 = ml_dtypes.bfloat16
    x = np.asarray(x, dtype=np.float32)
    pos = np.asarray(pos, dtype=np.float32)
    W_kv = np.asarray(W_kv, dtype=np.float32)
    W_out = np.asarray(W_out, dtype=np.float32)
    b_out = np.asarray(b_out, dtype=np.float32)
    w_pos = np.asarray(w_pos, dtype=np.float32)

    wkvT = np.ascontiguousarray(W_kv.T.astype(bf16))  # (512, 1024)
    wout = np.ascontiguousarray(W_out.T.astype(bf16))  # (512, 512)
    boutr = np.ascontiguousarray(b_out.reshape(1, DIM).astype(bf16))
    wposr = np.ascontiguousarray(
        np.broadcast_to(w_pos.astype(bf16), (128, IQ, POS_DIM))
    )

    in_maps = []
    for c in range(NC):
        g, s = c // 4, c % 4
        bs = slice(4 * g, 4 * g + BPC)
        isl = slice(s * IQ, (s + 1) * IQ)
        # roll tokens so this core's query quarter is columns 0:IQ
        xr = np.roll(x[bs], -s * IQ, axis=1)  # (4, 1024, 512)
        xT = np.ascontiguousarray(xr.transpose(0, 2, 1).astype(bf16))  # (4,512,1024)
        posr = np.roll(pos[0, isl, :, :], -s * IQ, axis=1)  # (256 i, 1024 j, 50)
        posT = np.ascontiguousarray(
            posr.transpose(1, 0, 2).reshape(JT, 128, IQ, POS_DIM).astype(bf16)
        )
        in_maps.append(
            {
                "xT": xT,
                "wkvT": wkvT,
                "wout": wout,
                "bout": boutr,
                "wposr": wposr,
                "posT": posT,
            }
        )
    return in_maps


def kernel(**inputs) -> np.ndarray:
    nc = _get_program()
    in_maps = _host_shard(**inputs)
    res = run_bass_kernel_spmd(nc, in_maps, list(range(NC)))
    out = np.empty((B, N, DIM), dtype=np.float32)
    for c in range(NC):
        g, s = c // 4, c % 4
        out[4 * g : 4 * g + BPC, s * IQ : (s + 1) * IQ, :] = res.results[c]["y"]
    return out


if __name__ == "__main__":
    import reference

    inputs = {k: np.asarray(v) for k, v in reference.setup_inputs().items()}
    expected = np.asarray(reference.reference(**inputs))
    actual = kernel(**inputs)
    err = np.abs(actual - expected).max()
    rel = err / np.abs(expected).max()
    print(f"absmax err: {err:.3e}  rel: {rel:.3e}")


# revision 15
# speedup vs baseline: 1.4974x; 1.0212x over previous
"""Trainium2 Bass kernel for nn_Attention_KV (dense transformer attention
with K=Q sharing and a linear positional bias), distributed over 8 cores.

Sharding: 2 batch-groups x 4 query-quarters (collective-free). Core
c = 4*g + s owns batches 4g..4g+3 and query rows i in [256*s, 256*(s+1)).
The positional bias pos_bias(i,j) is head/batch independent but sharded
by i-quarter, so each core loads exactly the pos slice it consumes.

Layout tricks (all pure host-side layout, no host math):
  - The token (j) axis is ROLLED per core so the core's own query
    quarter occupies columns 0:IQ of k^T. The query block is then just a
    slice of kT - no separate x_q input, no separate k_q matmuls - while
    keeping the program identical across cores (SPMD). j only ever
    appears inside sums, so the roll is invisible in the output.
  - pos is pre-transposed to [jt, j, i, p] blocks so each j-tile loads
    as one fully contiguous 3.2MB HBM stream.

All attention math keeps scores TRANSPOSED (keys j on partitions,
queries i on the free axis); dots = k @ k^T is symmetric so this is
free, and softmax + the attn @ v contraction need no on-chip transpose:
  - scores^T = c*dots lands in PSUM (c = scale*sum(w_pos))
  - es = Exp(scale=c)(dots) * P where P = exp(pos_bias^T) is computed
    ONCE per core (instead of re-adding pos via identity matmuls for
    every batch*head)
  - attn@v as lhsT = v_ext (ones column appended -> row 64 of the
    result is the softmax denominator Z), rhs = es
  - 1/Z = exp(-ln Z) on the Scalar engine (both functions live in one
    ACT table set), broadcast across partitions by GpSimd, folded into
    the PSUM->SBUF copy of U

Scheduling: engines execute their instruction streams IN ORDER, so PE
gaps in the attention phase (waiting on exp/mult of the scores) are
filled STATICALLY by weaving the next-next batch's kv matmul chunks
between attention heads (kv uses a third buffer set so no WAR stall),
and by interleaving the last two batches' attention head-by-head. This
keeps the PE busy-window dense enough to hold the HAM clock gate at
full rate. The pos-bias multiply is split DVE/GpSimd to halve the
vector-engine serial load. Everything flows in bf16 (f32 PSUM
accumulation). b_pos (a scalar added to every score) is dropped:
softmax is shift invariant.
"""

import sys

sys.path.insert(0, "/opt/trn_rl_repo")

import numpy as np

import concourse.bacc as bacc
import concourse.bass as bass
import concourse.mybir as mybir
from concourse import tile
from concourse.bass_utils import run_bass_kernel_spmd

B, N, DIM, H, POS_DIM = 8, 1024, 512, 8, 50
D = DIM // H  # 64
NC = 8  # cores
BPC = 4  # batches per core
IQ = 256  # query rows per core
JT = N // 128  # 8 j-tiles
SCALE = float(DIM) ** -0.5

F32 = mybir.dt.float32
F32R = mybir.dt.float32r
BF16 = mybir.dt.bfloat16
AX = mybir.AxisListType
ALU = mybir.AluOpType
ACTF = mybir.ActivationFunctionType

POS_CHUNK = 64  # i-columns of pos per multiply op
KVSETS = 3  # kv buffer sets: kv(b+2) streams while attn(b), attn(b+1) run


def build_program(reps: int = 1):
    nc = bacc.Bacc("TRN2", target_bir_lowering=False, debug=False)

    # ---- DRAM parameters (per-core) ----
    xT_d = nc.declare_dram_parameter("xT", [BPC, DIM, N], BF16, isOutput=False)
    wkvT_d = nc.declare_dram_parameter("wkvT", [DIM, 2 * DIM], BF16, isOutput=False)
    wout_d = nc.declare_dram_parameter("wout", [DIM, DIM], BF16, isOutput=False)
    bout_d = nc.declare_dram_parameter("bout", [1, DIM], BF16, isOutput=False)
    wposr_d = nc.declare_dram_parameter(
        "wposr", [128, POS_CHUNK, POS_DIM], BF16, isOutput=False
    )
    posT_d = nc.declare_dram_parameter(
        "posT", [JT, 128, IQ, POS_DIM], BF16, isOutput=False
    )
    y_d = nc.declare_dram_parameter("y", [BPC, IQ, DIM], F32, isOutput=True)

    with tile.TileContext(nc) as tc:
        with (
            tc.tile_pool(name="persist", bufs=1) as pp,
            tc.tile_pool(name="pos_in", bufs=2) as pos_pool,
            tc.tile_pool(name="exps", bufs=4) as epool,
            tc.tile_pool(name="rzs", bufs=2) as rzpool,
            tc.tile_pool(name="outsb", bufs=2) as opool,
            tc.tile_pool(name="mm_ps", bufs=2, space="PSUM") as mmps,
            tc.tile_pool(name="dots_ps", bufs=2, space="PSUM") as dotsps,
            tc.tile_pool(name="up_ps", bufs=2, space="PSUM") as upps,
        ):
            for _rep in range(reps):
                # ---- preload small tensors + weights ----
                wposr = pp.tile([128, POS_CHUNK, POS_DIM], BF16, tag="wposr")
                nc.sync.dma_start(wposr[:], wposr_d[:])
                wkvT = [
                    pp.tile([128, 2 * DIM], BF16, name=f"wkvT{t}", tag=f"wkvT{t}")
                    for t in range(4)
                ]
                for t in range(4):
                    nc.sync.dma_start(wkvT[t][:], wkvT_d[t * 128 : (t + 1) * 128, :])
                wout = [
                    pp.tile([64, DIM], BF16, name=f"wout{h}", tag=f"wout{h}")
                    for h in range(H)
                ]
                for h in range(H):
                    nc.sync.dma_start(wout[h][:], wout_d[h * 64 : (h + 1) * 64, :])
                bout = pp.tile([1, DIM], BF16, tag="bout")
                nc.sync.dma_start(bout[:], bout_d[:])
                ones1 = pp.tile([1, 128], BF16, tag="ones1")
                nc.vector.memset(ones1[:], 1.0)

                # c = scale * sum(w_pos) on every partition (exp scale)
                c_ap = pp.tile([128, 1], F32, tag="c_ap")
                nc.vector.tensor_reduce(c_ap[:], wposr[:, 0, :], axis=AX.X, op=ALU.add)
                nc.scalar.mul(c_ap[:], c_ap[:], SCALE)

                # v_ext tiles (all kv buffer sets), ones column set once
                vext_sets = {
                    s3: [
                        pp.tile(
                            [128, H, D + 1],
                            BF16,
                            name=f"vext{t}_{s3}",
                            tag=f"vext{t}_{s3}",
                        )
                        for t in range(JT)
                    ]
                    for s3 in range(KVSETS)
                }
                for s3 in range(KVSETS):
                    for t in range(JT):
                        nc.vector.memset(vext_sets[s3][t][:, :, D : D + 1], 1.0)
                xT_sets = {
                    s3: [
                        pp.tile([128, N], BF16, name=f"xT{t}_{s3}", tag=f"xT{t}_{s3}")
                        for t in range(4)
                    ]
                    for s3 in range(KVSETS)
                }
                kT_sets = {
                    s3: [
                        pp.tile([128, N], BF16, name=f"kT{t}_{s3}", tag=f"kT{t}_{s3}")
                        for t in range(4)
                    ]
                    for s3 in range(KVSETS)
                }

                # P = exp(pos_bias^T) for this core's i-quarter, all j
                pbias = pp.tile([128, JT, IQ], BF16, tag="pbias")
                pexp = pp.tile([128, JT, IQ], BF16, tag="pexp")

                def emit_pos():
                    # pos-bias pipeline: the weight multiply is split
                    # DVE (jt 0-3, needed first) / GpSimd (jt 4-7, slower
                    # but otherwise idle); the X-reduce is DVE-only.
                    for jt in range(JT):
                        pt = pos_pool.tile(
                            [128, IQ, POS_DIM], BF16, name="pchunk", tag="pchunk"
                        )
                        nc.sync.dma_start(pt[:], posT_d[jt])
                        eng = nc.vector if jt < 4 else nc.gpsimd
                        for ic in range(IQ // POS_CHUNK):
                            sl = slice(ic * POS_CHUNK, (ic + 1) * POS_CHUNK)
                            eng.tensor_tensor(
                                pt[:, sl, :], pt[:, sl, :], wposr[:], op=ALU.mult
                            )
                        with nc.allow_low_precision(
                            reason="pos bias flows in bf16 by design"
                        ):
                            nc.vector.tensor_reduce(
                                pbias[:, jt, :], pt[:], axis=AX.X, op=ALU.add
                            )
                        nc.scalar.activation(
                            pexp[:, jt, :], pbias[:, jt, :], ACTF.Exp
                        )

                # ---- kv: x load + k^T / v_ext builds, split into chunks ----
                kv_tiles = {}

                def emit_kv_dma(b):
                    s3 = b % KVSETS
                    xT = xT_sets[s3]
                    for t in range(4):
                        nc.sync.dma_start(
                            xT[t][:], xT_d[b, t * 128 : (t + 1) * 128, :]
                        )
                    kv_tiles[b] = (kT_sets[s3], vext_sets[s3])

                def kv_chunks(b):
                    """8 closures, each ~8 matmuls: 4 kT column groups then
                    4 v_ext pairs. Woven between attention heads so the PE
                    always has independent ready work during softmax waits."""
                    s3 = b % KVSETS
                    xT, kT, vext = xT_sets[s3], kT_sets[s3], vext_sets[s3]

                    def kt_group(t):
                        def emit():
                            pss = [
                                mmps.tile([128, 512], F32, name="mmtile", tag="mm")
                                for _ in range(2)
                            ]
                            for dc in range(4):
                                for nchunk in range(2):
                                    nc.tensor.matmul(
                                        pss[nchunk][:],
                                        wkvT[dc][:, t * 128 : (t + 1) * 128],
                                        xT[dc][:, nchunk * 512 : (nchunk + 1) * 512],
                                        start=(dc == 0),
                                        stop=(dc == 3),
                                    )
                            for nchunk in range(2):
                                nc.vector.tensor_copy(
                                    kT[t][:, nchunk * 512 : (nchunk + 1) * 512],
                                    pss[nchunk][:],
                                )

                        return emit

                    def vext_pair(p):
                        def emit():
                            for nt in (2 * p, 2 * p + 1):
                                ps = mmps.tile(
                                    [128, 512], F32, name="mmtile", tag="mm"
                                )
                                for dc in range(4):
                                    nc.tensor.matmul(
                                        ps[:],
                                        xT[dc][:, nt * 128 : (nt + 1) * 128],
                                        wkvT[dc][:, DIM : 2 * DIM],
                                        start=(dc == 0),
                                        stop=(dc == 3),
                                    )
                                nc.scalar.copy(
                                    vext[nt][:, :, 0:D],
                                    ps[:].rearrange("p (h d) -> p h d", h=H),
                                )

                        return emit

                    return [kt_group(t) for t in range(4)] + [
                        vext_pair(p) for p in range(4)
                    ]

                def emit_kv(b):
                    emit_kv_dma(b)
                    for chunk in kv_chunks(b):
                        chunk()

                # ---- attention ----
                UT_sets = {
                    s2: [
                        pp.tile([64, IQ], BF16, name=f"UT{h}_{s2}", tag=f"UT{h}_{s2}")
                        for h in range(H)
                    ]
                    for s2 in (0, 1)
                }

                def head_scores(b, h):
                    """dots + exp + pos multiply for both jt-groups; returns
                    the two es tiles. No up-matmuls yet - the caller weaves
                    independent PE work between scores and up."""
                    kT, _ = kv_tiles[b]
                    kt = kT[h // 2]
                    pr = slice(64 * (h % 2), 64 * (h % 2) + 64)
                    ess = []
                    for jg in range(JT // 4):
                        dots = dotsps.tile(
                            [128, 4 * IQ], F32, name="dotstile", tag="dots"
                        )
                        for q in range(4):
                            jt = jg * 4 + q
                            qsl = slice(q * IQ, (q + 1) * IQ)
                            nc.tensor.matmul(
                                dots[:, qsl],
                                kt[pr, jt * 128 : (jt + 1) * 128],
                                kt[pr, 0:IQ],
                                start=True,
                                stop=True,
                            )
                        es = epool.tile([128, 4 * IQ], BF16, name="expS", tag="expS")
                        nc.scalar.activation(es[:], dots[:], ACTF.Exp, scale=c_ap[:])
                        nc.vector.tensor_tensor(
                            es[:],
                            es[:],
                            pexp[:, jg * 4 : (jg + 1) * 4, :].rearrange(
                                "p a b -> p (a b)"
                            ),
                            op=ALU.mult,
                        )
                        ess.append(es)
                    return ess

                def head_up(b, h, ess):
                    """attn @ v_ext accumulation + softmax normalization."""
                    s2 = b % 2
                    _, vext = kv_tiles[b]
                    UT = UT_sets[s2]
                    up = upps.tile([D + 1, IQ], F32, name="uptile", tag="up")
                    for jg in range(JT // 4):
                        for q in range(4):
                            jt = jg * 4 + q
                            qsl = slice(q * IQ, (q + 1) * IQ)
                            nc.tensor.matmul(
                                up[:],
                                vext[jt][:, h, :],
                                ess[jg][:, qsl],
                                start=(jt == 0),
                                stop=(jt == JT - 1),
                            )
                    # row 64 of up = Z; normalize U while copying out. (An
                    # exp(-ln Z) ACT variant thrashes activation-table sets -
                    # Ln and Exp resolve to different table loads - so the
                    # multi-pass DVE reciprocal stays the cheapest option.)
                    rz = rzpool.tile([1, IQ], F32, name="rz", tag="rz")
                    nc.vector.reciprocal(rz[:], up[64:65, :])
                    rzb = rzpool.tile([64, IQ], F32, name="rzb", tag="rzb")
                    nc.gpsimd.partition_broadcast(rzb[:], rz[:])
                    nc.vector.tensor_tensor(UT[h][:], up[0:64, :], rzb[:], op=ALU.mult)

                def emit_attn_weave(b, kv_next=None):
                    """attention for batch b with the (b+2) kv chunks woven
                    into the PE stream right where the softmax waits are."""
                    fillers = kv_chunks(kv_next) if kv_next is not None else []
                    if kv_next is not None:
                        emit_kv_dma(kv_next)
                    for h in range(H):
                        ess = head_scores(b, h)
                        if h < len(fillers):
                            fillers[h]()
                        head_up(b, h, ess)

                def emit_attn_pair(b0, b1):
                    """attention for two batches, heads interleaved - each
                    batch's scores latency is hidden by the other's matmuls."""
                    for h in range(H):
                        ess0 = head_scores(b0, h)
                        ess1 = head_scores(b1, h)
                        head_up(b0, h, ess0)
                        head_up(b1, h, ess1)

                def emit_final(b):
                    s2 = b % 2
                    UT = UT_sets[s2]
                    for it in range(IQ // 128):
                        isl = slice(it * 128, (it + 1) * 128)
                        fps = mmps.tile([128, 512], F32, name="mmtile", tag="mm")
                        for h in range(H):
                            nc.tensor.matmul(
                                fps[:],
                                UT[h][:, isl],
                                wout[h][:],
                                start=(h == 0),
                                stop=False,
                            )
                        nc.tensor.matmul(
                            fps[:], ones1[:], bout[:], start=False, stop=True
                        )
                        ot = opool.tile([128, 512], F32, name="osb", tag="osb")
                        nc.vector.tensor_copy(ot[:], fps[:])
                        nc.sync.dma_start(y_d[b, isl, :], ot[:])

                emit_kv(0)
                emit_pos()
                emit_kv(1)
                emit_attn_weave(0, kv_next=2)
                emit_final(0)
                emit_attn_weave(1, kv_next=3)
                emit_final(1)
                emit_attn_pair(2, 3)
                emit_final(2)
                emit_final(3)

    nc.compile()
    return nc


_CACHE = {}


def _get_program():
    if "nc" not in _CACHE:
        _CACHE["nc"] = build_program()
    return _CACHE["nc"]


def _host_shard(x, pos, W_kv, W_out, b_out, w_pos, b_pos):
    """Build the 8 per-core input maps (pure layout work, no math)."""
    import ml_dtypes

    bf16 = ml_dtypes.bfloat16
    x = np.asarray(x, dtype=np.float32)
    pos = np.asarray(pos, dtype=np.float32)
    W_kv = np.asarray(W_kv, dtype=np.float32)
    W_out = np.asarray(W_out, dtype=np.float32)
    b_out = np.asarray(b_out, dtype=np.float32)
    w_pos = np.asarray(w_pos, dtype=np.float32)

    wkvT = np.ascontiguousarray(W_kv.T.astype(bf16))  # (512, 1024)
    wout = np.ascontiguousarray(W_out.T.astype(bf16))  # (512, 512)
    boutr = np.ascontiguousarray(b_out.reshape(1, DIM).astype(bf16))
    wposr = np.ascontiguousarray(
        np.broadcast_to(w_pos.astype(bf16), (128, POS_CHUNK, POS_DIM))
    )

    in_maps = []
    for c in range(NC):
        g, s = c // 4, c % 4
        bs = slice(4 * g, 4 * g + BPC)
        isl = slice(s * IQ, (s + 1) * IQ)
        # roll tokens so this core's query quarter is columns 0:IQ
        xr = np.roll(x[bs], -s * IQ, axis=1)  # (4, 1024, 512)
        xT = np.ascontiguousarray(xr.transpose(0, 2, 1).astype(bf16))  # (4,512,1024)
        posr = np.roll(pos[0, isl, :, :], -s * IQ, axis=1)  # (256 i, 1024 j, 50)
        posT = np.ascontiguousarray(
            posr.transpose(1, 0, 2).reshape(JT, 128, IQ, POS_DIM).astype(bf16)
        )
        in_maps.append(
            {
                "xT": xT,
                "wkvT": wkvT,
                "wout": wout,
                "bout": boutr,
                "wposr": wposr,
                "posT": posT,
            }
        )
    return in_maps


def kernel(**inputs) -> np.ndarray:
    nc = _get_program()
    in_maps = _host_shard(**inputs)
    res = run_bass_kernel_spmd(nc, in_maps, list(range(NC)))
    out = np.empty((B, N, DIM), dtype=np.float32)
    for c in range(NC):
        g, s = c // 4, c % 4
        out[4 * g : 4 * g + BPC, s * IQ : (s + 1) * IQ, :] = res.results[c]["y"]
    return out


if __name__ == "__main__":
    import reference

    inputs = {k: np.asarray(v) for k, v in reference.setup_inputs().items()}
    expected = np.asarray(reference.reference(**inputs))
    actual = kernel(**inputs)
    err = np.abs(actual - expected).max()
    rel = err / np.abs(expected).max()
    print(f"absmax err: {err:.3e}  rel: {rel:.3e}")


# revision 22
# speedup vs baseline: 1.7258x; 1.1526x over previous
"""Trainium2 Bass kernel for nn_Attention_KV (dense transformer attention
with K=Q sharing and a linear positional bias), distributed over 8 cores.

Sharding: 2 batch-groups x 4 query-quarters (collective-free). Core
c = 4*g + s owns batches 4g..4g+3 and query rows i in [256*s, 256*(s+1)).
The positional bias pos_bias(i,j) is head/batch independent but sharded
by i-quarter, so each core loads exactly the pos slice it consumes.

Layout tricks (all pure host-side layout, no host math):
  - The token (j) axis is ROLLED per core so the core's own query
    quarter occupies columns 0:IQ of k^T. The query block is then just a
    slice of kT - no separate x_q input, no separate k_q matmuls - while
    keeping the program identical across cores (SPMD). j only ever
    appears inside sums, so the roll is invisible in the output.
  - pos is pre-transposed to [jt, j, i, p] blocks so each j-tile loads
    as one fully contiguous 3.2MB HBM stream.

All attention math keeps scores TRANSPOSED (keys j on partitions,
queries i on the free axis); dots = k @ k^T is symmetric so this is
free, and softmax + the attn @ v contraction need no on-chip transpose:
  - scores^T = c*dots lands in PSUM (c = scale*sum(w_pos))
  - es = Exp(scale=c)(dots) * P where P = exp(pos_bias^T) is computed
    ONCE per core (instead of re-adding pos via identity matmuls for
    every batch*head)
  - attn@v as lhsT = v_ext (ones column appended -> row 64 of the
    result is the softmax denominator Z), rhs = es
  - 1/Z = exp(-ln Z) on the Scalar engine (both functions live in one
    ACT table set), broadcast across partitions by GpSimd, folded into
    the PSUM->SBUF copy of U

Scheduling: engines execute their instruction streams IN ORDER, so PE
gaps in the attention phase (waiting on exp/mult of the scores) are
filled STATICALLY by weaving the next-next batch's kv matmul chunks
between attention heads (kv uses a third buffer set so no WAR stall),
and by interleaving the last two batches' attention head-by-head. This
keeps the PE busy-window dense enough to hold the HAM clock gate at
full rate. The pos-bias multiply is split DVE/GpSimd to halve the
vector-engine serial load. Everything flows in bf16 (f32 PSUM
accumulation). b_pos (a scalar added to every score) is dropped:
softmax is shift invariant.
"""

import sys

sys.path.insert(0, "/opt/trn_rl_repo")

import numpy as np

import concourse.bacc as bacc
import concourse.bass as bass
import concourse.mybir as mybir
from concourse import tile
from concourse.bass_utils import run_bass_kernel_spmd

B, N, DIM, H, POS_DIM = 8, 1024, 512, 8, 50
D = DIM // H  # 64
NC = 8  # cores
BPC = 4  # batches per core
IQ = 256  # query rows per core
JT = N // 128  # 8 j-tiles
SCALE = float(DIM) ** -0.5

F32 = mybir.dt.float32
F32R = mybir.dt.float32r
BF16 = mybir.dt.bfloat16
AX = mybir.AxisListType
ALU = mybir.AluOpType
ACTF = mybir.ActivationFunctionType

POS_CHUNK = 64  # i-columns of pos per multiply op
KVSETS = 4  # all batches' kv resident: the whole kv build is PE filler
# work for the serial pos-bias pipeline that gates attention


def build_program(reps: int = 1):
    nc = bacc.Bacc("TRN2", target_bir_lowering=False, debug=False)

    # ---- DRAM parameters (per-core) ----
    xT_d = nc.declare_dram_parameter("xT", [BPC, DIM, N], BF16, isOutput=False)
    wkvT_d = nc.declare_dram_parameter("wkvT", [DIM, 2 * DIM], BF16, isOutput=False)
    wout_d = nc.declare_dram_parameter("wout", [DIM, DIM], BF16, isOutput=False)
    bout_d = nc.declare_dram_parameter("bout", [1, DIM], BF16, isOutput=False)
    wposr_d = nc.declare_dram_parameter(
        "wposr", [128, POS_CHUNK, POS_DIM], BF16, isOutput=False
    )
    posT_d = nc.declare_dram_parameter(
        "posT", [JT, 2, 128, 128, POS_DIM], BF16, isOutput=False
    )
    y_d = nc.declare_dram_parameter("y", [BPC, IQ, DIM], F32, isOutput=True)

    with tile.TileContext(nc) as tc:
        with (
            tc.tile_pool(name="persist", bufs=1) as pp,
            tc.tile_pool(name="pos_in", bufs=2) as pos_pool,
            tc.tile_pool(name="exps", bufs=6) as epool,
            tc.tile_pool(name="rzs", bufs=2) as rzpool,
            tc.tile_pool(name="outsb", bufs=2) as opool,
            tc.tile_pool(name="mm_ps", bufs=2, space="PSUM") as mmps,
            tc.tile_pool(name="dots_ps", bufs=2, space="PSUM") as dotsps,
            tc.tile_pool(name="up_ps", bufs=2, space="PSUM") as upps,
        ):
            for _rep in range(reps):
                # ---- preload small tensors + weights ----
                wposr = pp.tile([128, POS_CHUNK, POS_DIM], BF16, tag="wposr")
                nc.sync.dma_start(wposr[:], wposr_d[:])
                wkvT = [
                    pp.tile([128, 2 * DIM], BF16, name=f"wkvT{t}", tag=f"wkvT{t}")
                    for t in range(4)
                ]
                for t in range(4):
                    nc.sync.dma_start(wkvT[t][:], wkvT_d[t * 128 : (t + 1) * 128, :])
                wout = [
                    pp.tile([64, DIM], BF16, name=f"wout{h}", tag=f"wout{h}")
                    for h in range(H)
                ]
                for h in range(H):
                    nc.sync.dma_start(wout[h][:], wout_d[h * 64 : (h + 1) * 64, :])
                bout = pp.tile([1, DIM], BF16, tag="bout")
                nc.sync.dma_start(bout[:], bout_d[:])
                ones1 = pp.tile([1, 128], BF16, tag="ones1")
                nc.vector.memset(ones1[:], 1.0)

                # c = scale * sum(w_pos) on every partition (exp scale)
                c_ap = pp.tile([128, 1], F32, tag="c_ap")
                nc.vector.tensor_reduce(c_ap[:], wposr[:, 0, :], axis=AX.X, op=ALU.add)
                nc.scalar.mul(c_ap[:], c_ap[:], SCALE)

                # v_ext tiles (all kv buffer sets), ones column set once
                vext_sets = {
                    s3: [
                        pp.tile(
                            [128, H, D + 1],
                            BF16,
                            name=f"vext{t}_{s3}",
                            tag=f"vext{t}_{s3}",
                        )
                        for t in range(JT)
                    ]
                    for s3 in range(KVSETS)
                }
                for s3 in range(KVSETS):
                    for t in range(JT):
                        nc.vector.memset(vext_sets[s3][t][:, :, D : D + 1], 1.0)
                xT_sets = {
                    s3: [
                        pp.tile([128, N], BF16, name=f"xT{t}_{s3}", tag=f"xT{t}_{s3}")
                        for t in range(4)
                    ]
                    for s3 in range(KVSETS)
                }
                kT_sets = {
                    s3: [
                        pp.tile([128, N], BF16, name=f"kT{t}_{s3}", tag=f"kT{t}_{s3}")
                        for t in range(4)
                    ]
                    for s3 in range(KVSETS)
                }

                # P = exp(pos_bias^T) for this core's i-quarter, all j
                pbias = pp.tile([128, JT, IQ], BF16, tag="pbias")
                pexp = pp.tile([128, JT, IQ], BF16, tag="pexp")

                def emit_pos_quarter(qq):
                    # pos-bias pipeline, 4 half-jt-tiles per call so it can
                    # be interleaved between the kv batches. The weight
                    # multiply alternates DVE / GpSimd per half-tile (the
                    # two engines chew one jt in parallel); the X-reduce is
                    # DVE-only.
                    for k in range(4 * qq, 4 * qq + 4):
                        jt, ih = k // 2, k % 2
                        pt = pos_pool.tile(
                            [128, 128, POS_DIM], BF16, name="pchunk", tag="pchunk"
                        )
                        nc.sync.dma_start(pt[:], posT_d[jt, ih])
                        eng = nc.vector if ih == 0 else nc.gpsimd
                        for ic in range(128 // POS_CHUNK):
                            sl = slice(ic * POS_CHUNK, (ic + 1) * POS_CHUNK)
                            eng.tensor_tensor(
                                pt[:, sl, :], pt[:, sl, :], wposr[:], op=ALU.mult
                            )
                        isl = slice(ih * 128, (ih + 1) * 128)
                        with nc.allow_low_precision(
                            reason="pos bias flows in bf16 by design"
                        ):
                            nc.vector.tensor_reduce(
                                pbias[:, jt, isl], pt[:], axis=AX.X, op=ALU.add
                            )
                        if ih == 1:
                            nc.scalar.activation(
                                pexp[:, jt, :], pbias[:, jt, :], ACTF.Exp
                            )

                # ---- kv: x load + k^T / v_ext builds, split into chunks ----
                kv_tiles = {}

                def emit_kv_dma(b):
                    s3 = b % KVSETS
                    xT = xT_sets[s3]
                    for t in range(4):
                        nc.sync.dma_start(
                            xT[t][:], xT_d[b, t * 128 : (t + 1) * 128, :]
                        )
                    kv_tiles[b] = (kT_sets[s3], vext_sets[s3])

                def kv_chunks(b):
                    """8 closures, each ~8 matmuls: 4 kT column groups then
                    4 v_ext pairs. Woven between attention heads so the PE
                    always has independent ready work during softmax waits."""
                    s3 = b % KVSETS
                    xT, kT, vext = xT_sets[s3], kT_sets[s3], vext_sets[s3]

                    def kt_group(t):
                        def emit():
                            pss = [
                                mmps.tile([128, 512], F32, name="mmtile", tag="mm")
                                for _ in range(2)
                            ]
                            for dc in range(4):
                                for nchunk in range(2):
                                    nc.tensor.matmul(
                                        pss[nchunk][:],
                                        wkvT[dc][:, t * 128 : (t + 1) * 128],
                                        xT[dc][:, nchunk * 512 : (nchunk + 1) * 512],
                                        start=(dc == 0),
                                        stop=(dc == 3),
                                    )
                            for nchunk in range(2):
                                nc.vector.tensor_copy(
                                    kT[t][:, nchunk * 512 : (nchunk + 1) * 512],
                                    pss[nchunk][:],
                                )

                        return emit

                    def vext_pair(p):
                        def emit():
                            for nt in (2 * p, 2 * p + 1):
                                ps = mmps.tile(
                                    [128, 512], F32, name="mmtile", tag="mm"
                                )
                                for dc in range(4):
                                    nc.tensor.matmul(
                                        ps[:],
                                        xT[dc][:, nt * 128 : (nt + 1) * 128],
                                        wkvT[dc][:, DIM : 2 * DIM],
                                        start=(dc == 0),
                                        stop=(dc == 3),
                                    )
                                nc.scalar.copy(
                                    vext[nt][:, :, 0:D],
                                    ps[:].rearrange("p (h d) -> p h d", h=H),
                                )

                        return emit

                    return [kt_group(t) for t in range(4)] + [
                        vext_pair(p) for p in range(4)
                    ]

                def emit_kv(b):
                    emit_kv_dma(b)
                    for chunk in kv_chunks(b):
                        chunk()

                # ---- attention ----
                UT_sets = {
                    s2: [
                        pp.tile([64, IQ], BF16, name=f"UT{h}_{s2}", tag=f"UT{h}_{s2}")
                        for h in range(H)
                    ]
                    for s2 in (0, 1)
                }

                def head_scores(b, h):
                    """dots + exp + pos multiply for both jt-groups; returns
                    the two es tiles. No up-matmuls yet - the caller weaves
                    independent PE work between scores and up."""
                    kT, _ = kv_tiles[b]
                    kt = kT[h // 2]
                    pr = slice(64 * (h % 2), 64 * (h % 2) + 64)
                    ess = []
                    for jg in range(JT // 4):
                        dots = dotsps.tile(
                            [128, 4 * IQ], F32, name="dotstile", tag="dots"
                        )
                        for q in range(4):
                            jt = jg * 4 + q
                            qsl = slice(q * IQ, (q + 1) * IQ)
                            nc.tensor.matmul(
                                dots[:, qsl],
                                kt[pr, jt * 128 : (jt + 1) * 128],
                                kt[pr, 0:IQ],
                                start=True,
                                stop=True,
                            )
                        es = epool.tile([128, 4 * IQ], BF16, name="expS", tag="expS")
                        nc.scalar.activation(es[:], dots[:], ACTF.Exp, scale=c_ap[:])
                        nc.vector.tensor_tensor(
                            es[:],
                            es[:],
                            pexp[:, jg * 4 : (jg + 1) * 4, :].rearrange(
                                "p a b -> p (a b)"
                            ),
                            op=ALU.mult,
                        )
                        ess.append(es)
                    return ess

                def head_up(b, h, ess):
                    """attn @ v_ext accumulation + softmax normalization."""
                    s2 = b % 2
                    _, vext = kv_tiles[b]
                    UT = UT_sets[s2]
                    up = upps.tile([D + 1, IQ], F32, name="uptile", tag="up")
                    for jg in range(JT // 4):
                        for q in range(4):
                            jt = jg * 4 + q
                            qsl = slice(q * IQ, (q + 1) * IQ)
                            nc.tensor.matmul(
                                up[:],
                                vext[jt][:, h, :],
                                ess[jg][:, qsl],
                                start=(jt == 0),
                                stop=(jt == JT - 1),
                            )
                    # row 64 of up = Z; normalize U while copying out. (An
                    # exp(-ln Z) ACT variant thrashes activation-table sets -
                    # Ln and Exp resolve to different table loads - so the
                    # multi-pass DVE reciprocal stays the cheapest option.)
                    rz = rzpool.tile([1, IQ], F32, name="rz", tag="rz")
                    nc.vector.reciprocal(rz[:], up[64:65, :])
                    rzb = rzpool.tile([64, IQ], F32, name="rzb", tag="rzb")
                    nc.gpsimd.partition_broadcast(rzb[:], rz[:])
                    nc.vector.tensor_tensor(UT[h][:], up[0:64, :], rzb[:], op=ALU.mult)

                def emit_attn_pair(b0, b1):
                    """attention for two batches, heads interleaved - each
                    batch's scores latency is hidden by the other's matmuls."""
                    for h in range(H):
                        ess0 = head_scores(b0, h)
                        ess1 = head_scores(b1, h)
                        head_up(b0, h, ess0)
                        head_up(b1, h, ess1)

                def emit_final(b):
                    s2 = b % 2
                    UT = UT_sets[s2]
                    for it in range(IQ // 128):
                        isl = slice(it * 128, (it + 1) * 128)
                        fps = mmps.tile([128, 512], F32, name="mmtile", tag="mm")
                        for h in range(H):
                            nc.tensor.matmul(
                                fps[:],
                                UT[h][:, isl],
                                wout[h][:],
                                start=(h == 0),
                                stop=False,
                            )
                        nc.tensor.matmul(
                            fps[:], ones1[:], bout[:], start=False, stop=True
                        )
                        ot = opool.tile([128, 512], F32, name="osb", tag="osb")
                        nc.vector.tensor_copy(ot[:], fps[:])
                        nc.sync.dma_start(y_d[b, isl, :], ot[:])

                emit_kv(0)
                emit_pos_quarter(0)
                emit_kv(1)
                emit_pos_quarter(1)
                emit_kv(2)
                emit_pos_quarter(2)
                emit_kv(3)
                emit_pos_quarter(3)
                emit_attn_pair(0, 1)
                emit_final(0)
                emit_final(1)
                emit_attn_pair(2, 3)
                emit_final(2)
                emit_final(3)

    nc.compile()
    return nc


_CACHE = {}


def _get_program():
    if "nc" not in _CACHE:
        _CACHE["nc"] = build_program()
    return _CACHE["nc"]


def _host_shard(x, pos, W_kv, W_out, b_out, w_pos, b_pos):
    """Build the 8 per-core input maps (pure layout work, no math)."""
    import ml_dtypes

    bf16 = ml_dtypes.bfloat16
    x = np.asarray(x, dtype=np.float32)
    pos = np.asarray(pos, dtype=np.float32)
    W_kv = np.asarray(W_kv, dtype=np.float32)
    W_out = np.asarray(W_out, dtype=np.float32)
    b_out = np.asarray(b_out, dtype=np.float32)
    w_pos = np.asarray(w_pos, dtype=np.float32)

    wkvT = np.ascontiguousarray(W_kv.T.astype(bf16))  # (512, 1024)
    wout = np.ascontiguousarray(W_out.T.astype(bf16))  # (512, 512)
    boutr = np.ascontiguousarray(b_out.reshape(1, DIM).astype(bf16))
    wposr = np.ascontiguousarray(
        np.broadcast_to(w_pos.astype(bf16), (128, POS_CHUNK, POS_DIM))
    )

    in_maps = []
    for c in range(NC):
        g, s = c // 4, c % 4
        bs = slice(4 * g, 4 * g + BPC)
        isl = slice(s * IQ, (s + 1) * IQ)
        # roll tokens so this core's query quarter is columns 0:IQ
        xr = np.roll(x[bs], -s * IQ, axis=1)  # (4, 1024, 512)
        xT = np.ascontiguousarray(xr.transpose(0, 2, 1).astype(bf16))  # (4,512,1024)
        posr = np.roll(pos[0, isl, :, :], -s * IQ, axis=1)  # (256 i, 1024 j, 50)
        posT = np.ascontiguousarray(
            posr.transpose(1, 0, 2)
            .reshape(JT, 128, 2, 128, POS_DIM)
            .transpose(0, 2, 1, 3, 4)  # (jt, i-half, j, i, p) blocks
            .astype(bf16)
        )
        in_maps.append(
            {
                "xT": xT,
                "wkvT": wkvT,
                "wout": wout,
                "bout": boutr,
                "wposr": wposr,
                "posT": posT,
            }
        )
    return in_maps


def kernel(**inputs) -> np.ndarray:
    nc = _get_program()
    in_maps = _host_shard(**inputs)
    res = run_bass_kernel_spmd(nc, in_maps, list(range(NC)))
    out = np.empty((B, N, DIM), dtype=np.float32)
    for c in range(NC):
        g, s = c // 4, c % 4
        out[4 * g : 4 * g + BPC, s * IQ : (s + 1) * IQ, :] = res.results[c]["y"]
    return out


if __name__ == "__main__":
    import reference

    inputs = {k: np.asarray(v) for k, v in reference.setup_inputs().items()}
    expected = np.asarray(reference.reference(**inputs))
    actual = kernel(**inputs)
    err = np.abs(actual - expected).max()
    rel = err / np.abs(expected).max()
    print(f"absmax err: {err:.3e}  rel: {rel:.3e}")
